# revision 30
# baseline (speedup 1.0000x reference)
"""Trainium2 Bass kernel for nn_DifferentiableFDN.

Math: the module is linear in x, so
    out[b,t] = sum_j w_j * y_j[b,t],   w = (H^T alpha + beta)/16,
    y_j = first-order IIR of x with decay a_j.

Blocked-scan scheme (chunk length L=128, NCH=375 chunks per batch row).
The host pre-transposes x into XT[b] = (t=128, c=375) and un-transposes the
output. All matmul operands are bf16 (PSUM accumulates fp32; the 2e-2 rel-err
gate leaves ~50x headroom); the chunk-carry scan state stays fp32 inside the
DVE. Per batch row:
  - e  = P^T  @ XT   (16 x 375)   chunk-end state contributions, written at
         partition offset 16b into one stacked PSUM tile (64 x 375)
  - S  : ONE tensor_tensor_scan over the stacked tile, S[c] = a_j^L S[c-1]+e[c],
         written bf16 into the shifted position ssh[c] = S[c-1]
  - z  = MT^T @ XT   (128 x 375)  local Toeplitz part  (PSUM, start)
  - z += Wc^T @ ssh  (128 x 375)  rank-16 carry correction (PSUM, stop)
  out[b, c*128+tp] = z[tp, c], staged to SBUF as bf16, host converts to f32.

Matmuls are grouped by stationary weights (P x4, MT x4, Wc x4) so the PE can
keep weights loaded. No warm-up matmuls: the kernel is shorter than the HAM
ramp, so the PE runs at the throttled clock either way, and every extra
Tensor-queue instruction costs ~115ns in the framework's end-of-kernel
semaphore clear (the dominant fixed tail).

Sharding: pure data-parallel, 4 batch rows per core x 8 cores.
"""
import numpy as np
import ml_dtypes

B, T = 32, 48000
D = 16
NCORES = 8
BL = B // NCORES            # 4 batch rows per core
L = 128                     # chunk length
NCH = T // L                # 375 chunks per batch row
CL = 188                    # left-half chunks (pipeline split)
CR = NCH - CL               # right-half chunks

_CACHE = {}


def _mirror_f32_params(log_kappa, alpha_raw, beta_raw, H):
    """Reference param math, f64 internally, rounded through f32 where the
    reference's f32 pipeline rounds."""
    sig = 1.0 / (1.0 + np.exp(-log_kappa.astype(np.float64)))
    sig32 = sig.astype(np.float32)
    kappa = (np.float32(1.0) + sig32 * np.float32(799.0)).astype(np.float32)
    inv = (np.float32(-1.0) / kappa).astype(np.float32)
    decays = np.exp(inv.astype(np.float64)).astype(np.float32)
    decays = np.clip(decays, 0.0, 0.9999).astype(np.float64)
    alpha = (1.0 / (1.0 + np.exp(-alpha_raw.astype(np.float64))))
    beta = (1.0 / (1.0 + np.exp(-beta_raw.astype(np.float64))))
    alpha = alpha.astype(np.float32).astype(np.float64)
    beta = beta.astype(np.float32).astype(np.float64)
    w = (H.astype(np.float64).T @ alpha + beta) / np.float64(D)
    return decays, w


def _tables(decays, w):
    delta = np.arange(L)
    pows = decays[None, :] ** delta[:, None]                   # [L, D] a_j^d
    h = pows @ w                                               # h[d]
    MT = np.zeros((L, L))
    for t in range(L):
        MT[t, t:] = h[: L - t]                                 # MT[t,tp]=h[tp-t]
    P = decays[None, :] ** (L - 1 - delta[:, None])            # [L, D]
    Wc = w[:, None] * decays[:, None] ** (delta[None, :] + 1)  # [D, L]
    bf = ml_dtypes.bfloat16
    # cc = [MT | P | Wc-replicated] (128 x 272) bf16, one DMA.
    # The 4 batch rows' chunk-end states live at PSUM partition offsets
    # 0/32/64/96 (the only legal PE output tile positions), so the corr
    # weights Wc and the scan multiplier mlc (128 x 1, f32 — the scan state
    # is fp32) are replicated at those offsets.
    cc = np.zeros((L, 272), dtype=bf)
    cc[:, 0:128] = MT.astype(bf)
    cc[:, 128:144] = P.astype(bf)
    mlc = np.zeros((L, 1), dtype=np.float32)
    for b in range(BL):
        cc[32 * b:32 * b + D, 144:272] = Wc.astype(bf)
        mlc[32 * b:32 * b + D, 0] = (decays ** L).astype(np.float32)
    return np.ascontiguousarray(cc), np.ascontiguousarray(mlc)


def _body(tc, o_ap, x_ap, cc_ap, ml_ap):
    from concourse import mybir
    from contextlib import ExitStack

    nc = tc.nc
    f32 = mybir.dt.float32
    bf16 = mybir.dt.bfloat16

    with ExitStack() as ctx:
        const = ctx.enter_context(tc.tile_pool(name="const", bufs=1))
        xtp = ctx.enter_context(tc.tile_pool(name="xt", bufs=1))
        sshp = ctx.enter_context(tc.tile_pool(name="sshp", bufs=1))
        stgp = ctx.enter_context(tc.tile_pool(name="stg", bufs=1))
        epp = ctx.enter_context(tc.tile_pool(name="e_ps", bufs=1, space="PSUM"))
        zpp = ctx.enter_context(tc.tile_pool(name="z_ps", bufs=1, space="PSUM"))

        cc = const.tile([L, 272], bf16, tag="cc")
        mlc = const.tile([L, 1], f32, tag="mlc")
        # batch rows are PAIRED per SBUF tile: 1500B partition lines keep the
        # DMA queues at full rate (750B lines run at ~half throughput)
        xtq = [xtp.tile([L, 2 * NCH], bf16, tag=f"xt{q}", name=f"xt{q}")
               for q in range(2)]
        xt = [xtq[b // 2][:, (b % 2) * NCH:(b % 2 + 1) * NCH] for b in range(BL)]
        ssh = sshp.tile([L, NCH], bf16, tag="ssh")
        e_all = epp.tile([L, NCH], f32, tag="e")

        # input DMAs: two HW queues (sync=SP, scalar=Activation).  The
        # scalar queue starts ~0.7us late (its activation-table fetch rides
        # the queue first), so xt01 leads the fast sync queue and xt23
        # trails the const pack on the scalar queue.  The resulting stagger
        # lets the scheduler run Z0/Z1 in the otherwise-idle PE window
        # before the E quad, hiding the serial full-array Z matmuls behind
        # the scan.
        nc.sync.dma_start(xtq[0][:, :], x_ap[0:L, :])
        nc.sync.dma_start(mlc[:, :], ml_ap[:, :])
        nc.scalar.dma_start(cc[:, :], cc_ap[:, :])
        nc.scalar.dma_start(xtq[1][:, :], x_ap[L:2 * L, :])

        # scan writes cols 1..NCH-1; col 0 is the zero initial state
        nc.gpsimd.memset(ssh[:, 0:1], 0.0)

        mt_sb, p_sb = cc[:, 0:128], cc[:, 128:144]

        # chunk-end states: 4 matmuls, same stationary P, partition-offset
        # writes (tile positions 0/32/64/96) into one stacked PSUM tile;
        # disjoint column quadrants let all four run concurrently on the PE
        for b in range(BL):
            nc.tensor.matmul(e_all[32 * b:32 * b + D, :], lhsT=p_sb,
                             rhs=xt[b], start=True, stop=True,
                             skip_group_check=True, tile_position=(0, 32 * b))

        # The scan, carry correction, staging copies and output DMAs are all
        # split into column halves (L: chunks 0..CL-1, R: the rest) so the
        # L half drains while the R half is still being computed.  The R
        # scan chains off the L scan's final state (ssh[:, CL] = S[CL-1]).
        nc.vector.tensor_tensor_scan(
            ssh[:, 1:CL + 1], data0=mlc[:, 0:1].broadcast_to((L, CL)),
            data1=e_all[:, 0:CL],
            initial=0.0, op0=mybir.AluOpType.mult, op1=mybir.AluOpType.add)
        nc.vector.tensor_tensor_scan(
            ssh[:, CL + 1:NCH], data0=mlc[:, 0:1].broadcast_to((L, CR - 1)),
            data1=e_all[:, CL:NCH - 1],
            initial=ssh[:, CL:CL + 1],
            op0=mybir.AluOpType.mult, op1=mybir.AluOpType.add)

        # Z stays one matmul per bank: start=True zeroes the ENTIRE bank row
        # of every partition it writes, so a split Z would wipe its sibling
        # half.  The corr matmuls accumulate (start=False) and are safe to
        # split into column halves.
        z = [zpp.tile([L, NCH], f32, tag=f"z{b}", name=f"z{b}")
             for b in range(BL)]
        for b in range(BL):
            nc.tensor.matmul(z[b][:, :], lhsT=mt_sb, rhs=xt[b][:, :],
                             start=True, stop=False, skip_group_check=True)
        for b in range(BL):
            nc.tensor.matmul(z[b][:, 0:CL], lhsT=cc[32 * b:32 * b + D, 144:272],
                             rhs=ssh[32 * b:32 * b + D, 0:CL],
                             start=False, stop=False, skip_group_check=True,
                             tile_position=(32 * b, 0))
        for b in range(BL):
            nc.tensor.matmul(z[b][:, CL:NCH], lhsT=cc[32 * b:32 * b + D, 144:272],
                             rhs=ssh[32 * b:32 * b + D, CL:NCH],
                             start=False, stop=True, skip_group_check=True,
                             tile_position=(32 * b, 0))

        # staging: one [128, 4*CL] L tile and one [128, 4*CR] R tile (block
        # layout keeps DMA lines >= 1496B); DVE copies even b, Act odd b
        stL = stgp.tile([L, BL * CL], bf16, tag="stL", name="stL")
        stR = stgp.tile([L, BL * CR], bf16, tag="stR", name="stR")
        for b in range(BL):
            dstL = stL[:, b * CL:(b + 1) * CL]
            dstR = stR[:, b * CR:(b + 1) * CR]
            if b % 2:
                nc.scalar.copy(dstL, z[b][:, 0:CL])
            else:
                nc.vector.tensor_copy(dstL, z[b][:, 0:CL])
        for b in range(BL):
            dstR = stR[:, b * CR:(b + 1) * CR]
            if b % 2:
                nc.scalar.copy(dstR, z[b][:, CL:NCH])
            else:
                nc.vector.tensor_copy(dstR, z[b][:, CL:NCH])
        nc.sync.dma_start(o_ap[:, 0:BL * CL], stL[:, :])
        nc.scalar.dma_start(o_ap[:, BL * CL:BL * NCH], stR[:, :])


def _build(num_devices=NCORES):
    import concourse.tile as tile
    from concourse import bacc, mybir

    f32 = mybir.dt.float32
    bf16 = mybir.dt.bfloat16
    nc = bacc.Bacc("TRN2", target_bir_lowering=False, debug=False,
                   num_devices=num_devices)
    # x rows 0..127 = queue 0 (b0|b1 column-paired), rows 128..255 = queue 1
    x_ap = nc.dram_tensor("x", [2 * L, 2 * NCH], bf16, kind="ExternalInput").ap()
    cc_ap = nc.dram_tensor("cc", [L, 272], bf16, kind="ExternalInput").ap()
    ml_ap = nc.dram_tensor("mlc", [L, 1], f32, kind="ExternalInput").ap()
    # out[tp, b*NCH + c]
    o_ap = nc.dram_tensor("out", [L, BL * NCH], bf16, kind="ExternalOutput").ap()

    with tile.TileContext(nc) as tc:
        _body(tc, o_ap, x_ap, cc_ap, ml_ap)
    nc.compile()
    return nc


def _in_maps(x, log_kappa, alpha_raw, beta_raw, H):
    decays, w = _mirror_f32_params(np.asarray(log_kappa), np.asarray(alpha_raw),
                                   np.asarray(beta_raw), np.asarray(H))
    cc, mlc = _tables(decays, w)
    bf = ml_dtypes.bfloat16
    x = np.asarray(x, dtype=np.float32)
    # host pre-transpose: (B, T) -> per-core (2*L, 2*NCH) with batch rows
    # column-paired per DMA queue, bf16
    xt_all = x.reshape(B, NCH, L).transpose(0, 2, 1).astype(bf)  # (B, L, NCH)
    maps = []
    for c in range(NCORES):
        quad = xt_all[c * BL:(c + 1) * BL]           # (4, L, NCH)
        xs = quad.reshape(2, 2, L, NCH).transpose(0, 2, 1, 3).reshape(
            2 * L, 2 * NCH)                          # row q*L+p, col b*NCH+c
        maps.append({"x": np.ascontiguousarray(xs), "cc": cc, "mlc": mlc})
    return maps


def _gather(results):
    # out dram per core: (L, BL*NCH) = [tp, (L-block: b*CL+c | R-block:
    # b*CR+(c-CL))] -> (BL, T), t = c*L + tp
    outs = []
    for c in range(NCORES):
        arr = np.asarray(results[c]["out"])
        lhs = arr[:, :BL * CL].reshape(L, BL, CL)
        rhs = arr[:, BL * CL:].reshape(L, BL, CR)
        full = np.concatenate([lhs, rhs], axis=2)       # (L, BL, NCH)
        outs.append(full.transpose(1, 2, 0).reshape(BL, T))
    return np.concatenate(outs, axis=0).astype(np.float32)


def kernel(x, log_kappa, alpha_raw, beta_raw, H):
    from concourse import bass_utils

    if "nc" not in _CACHE:
        _CACHE["nc"] = _build()
    nc = _CACHE["nc"]
    maps = _in_maps(x, log_kappa, alpha_raw, beta_raw, H)
    res = bass_utils.run_bass_kernel_spmd(nc, maps, core_ids=list(range(NCORES)))
    return _gather(res.results)


# revision 31
# speedup vs baseline: 1.0141x; 1.0141x over previous
"""Trainium2 Bass kernel for nn_DifferentiableFDN.

Math: the module is linear in x, so
    out[b,t] = sum_j w_j * y_j[b,t],   w = (H^T alpha + beta)/16,
    y_j = first-order IIR of x with decay a_j.

Blocked-scan scheme (chunk length L=128, NCH=375 chunks per batch row).
The host pre-transposes x into XT[b] = (t=128, c=375) and un-transposes the
output. All matmul operands are bf16 (PSUM accumulates fp32; the 2e-2 rel-err
gate leaves ~50x headroom); the chunk-carry scan state stays fp32 inside the
DVE. Per batch row:
  - e  = P^T  @ XT   (16 x 375)   chunk-end state contributions, written at
         partition offset 16b into one stacked PSUM tile (64 x 375)
  - S  : ONE tensor_tensor_scan over the stacked tile, S[c] = a_j^L S[c-1]+e[c],
         written bf16 into the shifted position ssh[c] = S[c-1]
  - z  = MT^T @ XT   (128 x 375)  local Toeplitz part  (PSUM, start)
  - z += Wc^T @ ssh  (128 x 375)  rank-16 carry correction (PSUM, stop)
  out[b, c*128+tp] = z[tp, c], staged to SBUF as bf16, host converts to f32.

Matmuls are grouped by stationary weights (P x4, MT x4, Wc x4) so the PE can
keep weights loaded. No warm-up matmuls: the kernel is shorter than the HAM
ramp, so the PE runs at the throttled clock either way, and every extra
Tensor-queue instruction costs ~115ns in the framework's end-of-kernel
semaphore clear (the dominant fixed tail).

Sharding: pure data-parallel, 4 batch rows per core x 8 cores.
"""
import numpy as np
import ml_dtypes

B, T = 32, 48000
D = 16
NCORES = 8
BL = B // NCORES            # 4 batch rows per core
L = 128                     # chunk length
NCH = T // L                # 375 chunks per batch row

_CACHE = {}


def _mirror_f32_params(log_kappa, alpha_raw, beta_raw, H):
    """Reference param math, f64 internally, rounded through f32 where the
    reference's f32 pipeline rounds."""
    sig = 1.0 / (1.0 + np.exp(-log_kappa.astype(np.float64)))
    sig32 = sig.astype(np.float32)
    kappa = (np.float32(1.0) + sig32 * np.float32(799.0)).astype(np.float32)
    inv = (np.float32(-1.0) / kappa).astype(np.float32)
    decays = np.exp(inv.astype(np.float64)).astype(np.float32)
    decays = np.clip(decays, 0.0, 0.9999).astype(np.float64)
    alpha = (1.0 / (1.0 + np.exp(-alpha_raw.astype(np.float64))))
    beta = (1.0 / (1.0 + np.exp(-beta_raw.astype(np.float64))))
    alpha = alpha.astype(np.float32).astype(np.float64)
    beta = beta.astype(np.float32).astype(np.float64)
    w = (H.astype(np.float64).T @ alpha + beta) / np.float64(D)
    return decays, w


def _tables(decays, w):
    delta = np.arange(L)
    pows = decays[None, :] ** delta[:, None]                   # [L, D] a_j^d
    h = pows @ w                                               # h[d]
    MT = np.zeros((L, L))
    for t in range(L):
        MT[t, t:] = h[: L - t]                                 # MT[t,tp]=h[tp-t]
    P = decays[None, :] ** (L - 1 - delta[:, None])            # [L, D]
    Wc = w[:, None] * decays[:, None] ** (delta[None, :] + 1)  # [D, L]
    bf = ml_dtypes.bfloat16
    # cc = [MT | P | Wc-replicated] (128 x 272) bf16, one DMA.
    # The 4 batch rows' chunk-end states live at PSUM partition offsets
    # 0/32/64/96 (the only legal PE output tile positions), so the corr
    # weights Wc and the scan multiplier mlc (128 x 1, f32 — the scan state
    # is fp32) are replicated at those offsets.
    cc = np.zeros((L, 272), dtype=bf)
    cc[:, 0:128] = MT.astype(bf)
    cc[:, 128:144] = P.astype(bf)
    mlc = np.zeros((L, 1), dtype=np.float32)
    for b in range(BL):
        cc[32 * b:32 * b + D, 144:272] = Wc.astype(bf)
        mlc[32 * b:32 * b + D, 0] = (decays ** L).astype(np.float32)
    return np.ascontiguousarray(cc), np.ascontiguousarray(mlc)


def _body(tc, o_ap, x_ap, cc_ap, ml_ap):
    from concourse import mybir
    from contextlib import ExitStack

    nc = tc.nc
    f32 = mybir.dt.float32
    bf16 = mybir.dt.bfloat16

    with ExitStack() as ctx:
        const = ctx.enter_context(tc.tile_pool(name="const", bufs=1))
        xtp = ctx.enter_context(tc.tile_pool(name="xt", bufs=1))
        sshp = ctx.enter_context(tc.tile_pool(name="sshp", bufs=1))
        stgp = ctx.enter_context(tc.tile_pool(name="stg", bufs=1))
        epp = ctx.enter_context(tc.tile_pool(name="e_ps", bufs=1, space="PSUM"))
        zpp = ctx.enter_context(tc.tile_pool(name="z_ps", bufs=1, space="PSUM"))

        cc = const.tile([L, 272], bf16, tag="cc")
        mlc = const.tile([L, 1], f32, tag="mlc")
        # batch rows are PAIRED per SBUF tile: 1500B partition lines keep the
        # DMA queues at full rate (750B lines run at ~half throughput)
        xtq = [xtp.tile([L, 2 * NCH], bf16, tag=f"xt{q}", name=f"xt{q}")
               for q in range(2)]
        xt = [xtq[b // 2][:, (b % 2) * NCH:(b % 2 + 1) * NCH] for b in range(BL)]
        ssh = sshp.tile([L, NCH], bf16, tag="ssh")
        e_all = epp.tile([L, NCH], f32, tag="e")

        # input DMAs: two HW queues (sync=SP, scalar=Activation).  The
        # scalar queue starts ~0.7us late (its activation-table fetch rides
        # the queue first), so xt01 leads the fast sync queue and xt23
        # trails the const pack on the scalar queue.  The resulting stagger
        # lets the scheduler run Z0/Z1 in the otherwise-idle PE window
        # before the E quad, hiding the serial full-array Z matmuls behind
        # the scan.
        nc.sync.dma_start(xtq[0][:, :], x_ap[0:L, :])
        nc.sync.dma_start(mlc[:, :], ml_ap[:, :])
        nc.scalar.dma_start(cc[:, :], cc_ap[:, :])
        nc.scalar.dma_start(xtq[1][:, :], x_ap[L:2 * L, :])

        # scan writes cols 1..NCH-1; col 0 is the zero initial state
        nc.gpsimd.memset(ssh[:, 0:1], 0.0)

        mt_sb, p_sb = cc[:, 0:128], cc[:, 128:144]

        # chunk-end states: 4 matmuls, same stationary P, partition-offset
        # writes (tile positions 0/32/64/96) into one stacked PSUM tile;
        # disjoint column quadrants let all four run concurrently on the PE
        for b in range(BL):
            nc.tensor.matmul(e_all[32 * b:32 * b + D, :], lhsT=p_sb,
                             rhs=xt[b], start=True, stop=True,
                             skip_group_check=True, tile_position=(0, 32 * b))

        # ONE scan for all 4 batch rows (DVE cost is per-column, not
        # per-partition); fp32 state internally, bf16 output. The gap
        # partitions carry garbage that nothing reads.
        nc.vector.tensor_tensor_scan(
            ssh[:, 1:NCH], data0=mlc[:, 0:1].broadcast_to((L, NCH - 1)),
            data1=e_all[:, 0:NCH - 1],
            initial=0.0, op0=mybir.AluOpType.mult, op1=mybir.AluOpType.add)

        z = [zpp.tile([L, NCH], f32, tag=f"z{b}", name=f"z{b}")
             for b in range(BL)]
        for b in range(BL):
            nc.tensor.matmul(z[b][:, :], lhsT=mt_sb, rhs=xt[b][:, :],
                             start=True, stop=False, skip_group_check=True)
        for b in range(BL):
            nc.tensor.matmul(z[b][:, :], lhsT=cc[32 * b:32 * b + D, 144:272],
                             rhs=ssh[32 * b:32 * b + D, :],
                             start=False, stop=True, skip_group_check=True,
                             tile_position=(32 * b, 0))

        # staging is paired too (1500B lines, 2 output DMAs); within a pair
        # one copy runs on the DVE and one on the Activation engine
        stq = [stgp.tile([L, 2 * NCH], bf16, tag=f"stg{q}", name=f"stg{q}")
               for q in range(2)]
        for b in range(BL):
            dst = stq[b // 2][:, (b % 2) * NCH:(b % 2 + 1) * NCH]
            if b % 2:
                nc.scalar.copy(dst, z[b][:, :])
            else:
                nc.vector.tensor_copy(dst, z[b][:, :])
        nc.sync.dma_start(o_ap[:, 0:2 * NCH], stq[0][:, :])
        nc.scalar.dma_start(o_ap[:, 2 * NCH:4 * NCH], stq[1][:, :])


def _build(num_devices=NCORES):
    import concourse.tile as tile
    from concourse import bacc, mybir

    f32 = mybir.dt.float32
    bf16 = mybir.dt.bfloat16
    nc = bacc.Bacc("TRN2", target_bir_lowering=False, debug=False,
                   num_devices=num_devices)
    # x rows 0..127 = queue 0 (b0|b1 column-paired), rows 128..255 = queue 1
    x_ap = nc.dram_tensor("x", [2 * L, 2 * NCH], bf16, kind="ExternalInput").ap()
    cc_ap = nc.dram_tensor("cc", [L, 272], bf16, kind="ExternalInput").ap()
    ml_ap = nc.dram_tensor("mlc", [L, 1], f32, kind="ExternalInput").ap()
    # out[tp, b*NCH + c]
    o_ap = nc.dram_tensor("out", [L, BL * NCH], bf16, kind="ExternalOutput").ap()

    with tile.TileContext(nc) as tc:
        _body(tc, o_ap, x_ap, cc_ap, ml_ap)
    nc.compile()
    return nc


def _in_maps(x, log_kappa, alpha_raw, beta_raw, H):
    decays, w = _mirror_f32_params(np.asarray(log_kappa), np.asarray(alpha_raw),
                                   np.asarray(beta_raw), np.asarray(H))
    cc, mlc = _tables(decays, w)
    bf = ml_dtypes.bfloat16
    x = np.asarray(x, dtype=np.float32)
    # host pre-transpose: (B, T) -> per-core (2*L, 2*NCH) with batch rows
    # column-paired per DMA queue, bf16
    xt_all = x.reshape(B, NCH, L).transpose(0, 2, 1).astype(bf)  # (B, L, NCH)
    maps = []
    for c in range(NCORES):
        quad = xt_all[c * BL:(c + 1) * BL]           # (4, L, NCH)
        xs = quad.reshape(2, 2, L, NCH).transpose(0, 2, 1, 3).reshape(
            2 * L, 2 * NCH)                          # row q*L+p, col b*NCH+c
        maps.append({"x": np.ascontiguousarray(xs), "cc": cc, "mlc": mlc})
    return maps


def _gather(results):
    # out dram per core: (L, BL*NCH) = [tp, (b, c)] -> (BL, T), t = c*L + tp
    outs = []
    for c in range(NCORES):
        arr = np.asarray(results[c]["out"]).reshape(L, BL, NCH)
        outs.append(arr.transpose(1, 2, 0).reshape(BL, T))
    return np.concatenate(outs, axis=0).astype(np.float32)


def kernel(x, log_kappa, alpha_raw, beta_raw, H):
    from concourse import bass_utils

    if "nc" not in _CACHE:
        _CACHE["nc"] = _build()
    nc = _CACHE["nc"]
    maps = _in_maps(x, log_kappa, alpha_raw, beta_raw, H)
    res = bass_utils.run_bass_kernel_spmd(nc, maps, core_ids=list(range(NCORES)))
    return _gather(res.results)


# revision 32
# speedup vs baseline: 1.0184x; 1.0043x over previous
"""Trainium2 Bass kernel for nn_DifferentiableFDN.

Math: the module is linear in x, so
    out[b,t] = sum_j w_j * y_j[b,t],   w = (H^T alpha + beta)/16,
    y_j = first-order IIR of x with decay a_j.

Blocked-scan scheme (chunk length L=128, NCH=375 chunks per batch row).
The host pre-transposes x into XT[b] = (t=128, c=375) and un-transposes the
output. All matmul operands are bf16 (PSUM accumulates fp32; the 2e-2 rel-err
gate leaves ~7x headroom at the measured 2.6e-3); the chunk-carry scan state
stays fp32 inside the DVE. Per batch row b:
  - e  = P^T  @ XT   (16 x 375)   chunk-end state contributions, written at
         PSUM partition offset 32b (tile positions 0/32/64/96) into ONE
         stacked tile; the four matmuls occupy disjoint PE column quadrants
         and run concurrently
  - S  : ONE tensor_tensor_scan over the stacked tile (cost is per-column,
         not per-partition), S[c] = a_j^L S[c-1] + e[c], written bf16 into
         the shifted position ssh[c] = S[c-1] (fp32 state internally, so
         bf16 output does not compound)
  - z  = MT^T @ XT   (128 x 375)  local Toeplitz part  (PSUM, start=True;
         start zeroes the whole bank row, so z is ONE matmul per bank)
  - z += Wc^T @ ssh  (128 x 375)  rank-16 carry correction; the four corr
         matmuls use disjoint row quadrants and run concurrently
  out[b, c*128+tp] = z[tp, c], cast to bf16 into paired staging tiles
  (1500B DMA lines), two output DMAs, host converts to f32.

Timing notes (per perfetto/ntff analysis): the framework preamble + final
semaphore-file clear are a fixed ~13.1us floor (a trivial kernel measures
that); the PE runs HAM-throttled (~0.79 col/ns bf16) because the kernel is
far shorter than the ~9us ramp, so warm-up matmuls don't pay.  The serial
full-array Z matmuls hide behind the input stagger + scan; the tail is
copies (only DVE + Act can read PSUM) + one DMA round trip.

Sharding: pure data-parallel, 4 batch rows per core x 8 cores.
"""
import numpy as np
import ml_dtypes

B, T = 32, 48000
D = 16
NCORES = 8
BL = B // NCORES            # 4 batch rows per core
L = 128                     # chunk length
NCH = T // L                # 375 chunks per batch row

_CACHE = {}


def _mirror_f32_params(log_kappa, alpha_raw, beta_raw, H):
    """Reference param math, f64 internally, rounded through f32 where the
    reference's f32 pipeline rounds."""
    sig = 1.0 / (1.0 + np.exp(-log_kappa.astype(np.float64)))
    sig32 = sig.astype(np.float32)
    kappa = (np.float32(1.0) + sig32 * np.float32(799.0)).astype(np.float32)
    inv = (np.float32(-1.0) / kappa).astype(np.float32)
    decays = np.exp(inv.astype(np.float64)).astype(np.float32)
    decays = np.clip(decays, 0.0, 0.9999).astype(np.float64)
    alpha = (1.0 / (1.0 + np.exp(-alpha_raw.astype(np.float64))))
    beta = (1.0 / (1.0 + np.exp(-beta_raw.astype(np.float64))))
    alpha = alpha.astype(np.float32).astype(np.float64)
    beta = beta.astype(np.float32).astype(np.float64)
    w = (H.astype(np.float64).T @ alpha + beta) / np.float64(D)
    return decays, w


def _tables(decays, w):
    delta = np.arange(L)
    pows = decays[None, :] ** delta[:, None]                   # [L, D] a_j^d
    h = pows @ w                                               # h[d]
    MT = np.zeros((L, L))
    for t in range(L):
        MT[t, t:] = h[: L - t]                                 # MT[t,tp]=h[tp-t]
    P = decays[None, :] ** (L - 1 - delta[:, None])            # [L, D]
    Wc = w[:, None] * decays[:, None] ** (delta[None, :] + 1)  # [D, L]
    bf = ml_dtypes.bfloat16
    # cc = [MT | P | Wc-replicated] (128 x 272) bf16, one DMA.
    # The 4 batch rows' chunk-end states live at PSUM partition offsets
    # 0/32/64/96 (the only legal PE output tile positions), so the corr
    # weights Wc and the scan multiplier mlc (128 x 1, f32 — the scan state
    # is fp32) are replicated at those offsets.
    cc = np.zeros((L, 272), dtype=bf)
    cc[:, 0:128] = MT.astype(bf)
    cc[:, 128:144] = P.astype(bf)
    mlc = np.zeros((L, 1), dtype=np.float32)
    for b in range(BL):
        cc[32 * b:32 * b + D, 144:272] = Wc.astype(bf)
        mlc[32 * b:32 * b + D, 0] = (decays ** L).astype(np.float32)
    return np.ascontiguousarray(cc), np.ascontiguousarray(mlc)


def _body(tc, o_ap, x_ap, cc_ap, ml_ap):
    from concourse import mybir
    from contextlib import ExitStack

    nc = tc.nc
    f32 = mybir.dt.float32
    bf16 = mybir.dt.bfloat16

    with ExitStack() as ctx:
        const = ctx.enter_context(tc.tile_pool(name="const", bufs=1))
        xtp = ctx.enter_context(tc.tile_pool(name="xt", bufs=1))
        sshp = ctx.enter_context(tc.tile_pool(name="sshp", bufs=1))
        stgp = ctx.enter_context(tc.tile_pool(name="stg", bufs=1))
        epp = ctx.enter_context(tc.tile_pool(name="e_ps", bufs=1, space="PSUM"))
        zpp = ctx.enter_context(tc.tile_pool(name="z_ps", bufs=1, space="PSUM"))

        cc = const.tile([L, 272], bf16, tag="cc")
        mlc = const.tile([L, 1], f32, tag="mlc")
        # batch rows are PAIRED per SBUF tile: 1500B partition lines keep the
        # DMA queues at full rate (750B lines run at ~half throughput)
        xtq = [xtp.tile([L, 2 * NCH], bf16, tag=f"xt{q}", name=f"xt{q}")
               for q in range(2)]
        xt = [xtq[b // 2][:, (b % 2) * NCH:(b % 2 + 1) * NCH] for b in range(BL)]
        ssh = sshp.tile([L, NCH], bf16, tag="ssh")
        e_all = epp.tile([L, NCH], f32, tag="e")

        # input DMAs: two HW queues (sync=SP, scalar=Activation).  The
        # scalar queue starts ~0.7us late (its activation-table fetch rides
        # the queue first), so xt01 leads the fast sync queue and xt23
        # trails the const pack on the scalar queue.  The resulting stagger
        # lets the scheduler run Z0/Z1 in the otherwise-idle PE window
        # before the E quad, hiding the serial full-array Z matmuls behind
        # the scan.
        nc.sync.dma_start(xtq[0][:, :], x_ap[0:L, :])
        nc.sync.dma_start(mlc[:, :], ml_ap[:, :])
        nc.scalar.dma_start(cc[:, :], cc_ap[:, :])
        nc.scalar.dma_start(xtq[1][:, :], x_ap[L:2 * L, :])

        # scan writes cols 1..NCH-1; col 0 is the zero initial state
        nc.gpsimd.memset(ssh[:, 0:1], 0.0)

        mt_sb, p_sb = cc[:, 0:128], cc[:, 128:144]

        # chunk-end states: 4 matmuls, same stationary P, partition-offset
        # writes (tile positions 0/32/64/96) into one stacked PSUM tile;
        # disjoint column quadrants let all four run concurrently on the PE
        for b in range(BL):
            nc.tensor.matmul(e_all[32 * b:32 * b + D, :], lhsT=p_sb,
                             rhs=xt[b], start=True, stop=True,
                             skip_group_check=True, tile_position=(0, 32 * b))

        # ONE scan for all 4 batch rows (DVE cost is per-column, not
        # per-partition); fp32 state internally, bf16 output. The gap
        # partitions carry garbage that nothing reads.
        nc.vector.tensor_tensor_scan(
            ssh[:, 1:NCH], data0=mlc[:, 0:1].broadcast_to((L, NCH - 1)),
            data1=e_all[:, 0:NCH - 1],
            initial=0.0, op0=mybir.AluOpType.mult, op1=mybir.AluOpType.add)

        z = [zpp.tile([L, NCH], f32, tag=f"z{b}", name=f"z{b}")
             for b in range(BL)]
        for b in range(BL):
            nc.tensor.matmul(z[b][:, :], lhsT=mt_sb, rhs=xt[b][:, :],
                             start=True, stop=False, skip_group_check=True)
        for b in range(BL):
            nc.tensor.matmul(z[b][:, :], lhsT=cc[32 * b:32 * b + D, 144:272],
                             rhs=ssh[32 * b:32 * b + D, :],
                             start=False, stop=True, skip_group_check=True,
                             tile_position=(32 * b, 0))

        # staging is paired too (1500B lines, 2 output DMAs); within a pair
        # one copy runs on the DVE and one on the Activation engine
        stq = [stgp.tile([L, 2 * NCH], bf16, tag=f"stg{q}", name=f"stg{q}")
               for q in range(2)]
        for b in range(BL):
            dst = stq[b // 2][:, (b % 2) * NCH:(b % 2 + 1) * NCH]
            if b % 2:
                nc.scalar.copy(dst, z[b][:, :])
            else:
                nc.vector.tensor_copy(dst, z[b][:, :])
        nc.sync.dma_start(o_ap[:, 0:2 * NCH], stq[0][:, :])
        nc.scalar.dma_start(o_ap[:, 2 * NCH:4 * NCH], stq[1][:, :])


def _build(num_devices=NCORES):
    import concourse.tile as tile
    from concourse import bacc, mybir

    f32 = mybir.dt.float32
    bf16 = mybir.dt.bfloat16
    nc = bacc.Bacc("TRN2", target_bir_lowering=False, debug=False,
                   num_devices=num_devices)
    # x rows 0..127 = queue 0 (b0|b1 column-paired), rows 128..255 = queue 1
    x_ap = nc.dram_tensor("x", [2 * L, 2 * NCH], bf16, kind="ExternalInput").ap()
    cc_ap = nc.dram_tensor("cc", [L, 272], bf16, kind="ExternalInput").ap()
    ml_ap = nc.dram_tensor("mlc", [L, 1], f32, kind="ExternalInput").ap()
    # out[tp, b*NCH + c]
    o_ap = nc.dram_tensor("out", [L, BL * NCH], bf16, kind="ExternalOutput").ap()

    with tile.TileContext(nc) as tc:
        _body(tc, o_ap, x_ap, cc_ap, ml_ap)
    nc.compile()
    return nc


def _in_maps(x, log_kappa, alpha_raw, beta_raw, H):
    decays, w = _mirror_f32_params(np.asarray(log_kappa), np.asarray(alpha_raw),
                                   np.asarray(beta_raw), np.asarray(H))
    cc, mlc = _tables(decays, w)
    bf = ml_dtypes.bfloat16
    x = np.asarray(x, dtype=np.float32)
    # host pre-transpose: (B, T) -> per-core (2*L, 2*NCH) with batch rows
    # column-paired per DMA queue, bf16
    xt_all = x.reshape(B, NCH, L).transpose(0, 2, 1).astype(bf)  # (B, L, NCH)
    maps = []
    for c in range(NCORES):
        quad = xt_all[c * BL:(c + 1) * BL]           # (4, L, NCH)
        xs = quad.reshape(2, 2, L, NCH).transpose(0, 2, 1, 3).reshape(
            2 * L, 2 * NCH)                          # row q*L+p, col b*NCH+c
        maps.append({"x": np.ascontiguousarray(xs), "cc": cc, "mlc": mlc})
    return maps


def _gather(results):
    # out dram per core: (L, BL*NCH) = [tp, (b, c)] -> (BL, T), t = c*L + tp
    outs = []
    for c in range(NCORES):
        arr = np.asarray(results[c]["out"]).reshape(L, BL, NCH)
        outs.append(arr.transpose(1, 2, 0).reshape(BL, T))
    return np.concatenate(outs, axis=0).astype(np.float32)


def kernel(x, log_kappa, alpha_raw, beta_raw, H):
    from concourse import bass_utils

    if "nc" not in _CACHE:
        _CACHE["nc"] = _build()
    nc = _CACHE["nc"]
    maps = _in_maps(x, log_kappa, alpha_raw, beta_raw, H)
    res = bass_utils.run_bass_kernel_spmd(nc, maps, core_ids=list(range(NCORES)))
    return _gather(res.results)
